# revision 1
# baseline (speedup 1.0000x reference)
"""DiffuMamba forward on 8 trn2 NeuronCores (Bass/Tile).

Sharding:
  - cores 0-3 handle batch 0, cores 4-7 batch 1 (trunk replicated in group).
  - Mamba: each core owns BOTH scan directions on a DI/4 slice (384 channels);
    dtbc/delta projections are host-fused into D-> . weights so no cross-core
    contraction is needed before the scan. Wout partials are AllReduced over
    the 4-core batch group; same for the SwiGLU MLP (H/4 per core).
  - lm_head: tied embedding vocab-sharded 8 ways; normed final states are
    AllGathered across batch pairs; log-softmax uses exp+accum_out, an 8-core
    AllReduce of the per-token sums, then ln(e * (1/gsum)).
  - SUBS forcing (unmasked positions -> one-hot rows) via indirect scatters
    with bounds-check skipping; offsets precomputed on host.
"""
import sys as _sys
for _p in ('/opt/trn_rl_repo', '/opt/trn_rl_repo/concourse'):
    if _p not in _sys.path:
        _sys.path.insert(0, _p)

from contextlib import ExitStack

import numpy as np
import ml_dtypes

import concourse.bass as bass
import concourse.mybir as mybir
from concourse import tile
from concourse.bass_utils import run_bass_kernel_spmd
from concourse.tile_rust import add_dep_helper

f32 = mybir.dt.float32
bf16 = mybir.dt.bfloat16
i32 = mybir.dt.int32
AF = mybir.ActivationFunctionType
OP = mybir.AluOpType

B, L, V, D, NL = 2, 512, 50304, 768, 2
DI, N, DTR, H, COND, FREQ = 1536, 16, 48, 1536, 128, 256
MASK_ID = 50257
NCORES = 8
TOK = L
NKD = D // 128             # 6
NQ = DI // 4               # 384 per-core DI quarter per direction
NPT = NQ // 128            # 3
HQ = H // 4                # 384
VSH = V // NCORES          # 6288
VT_W = [512] * 12 + [144]
CH_N = 2                   # n's per scan chunk
NCH = N // CH_N            # 8
NEG = float(np.finfo(np.float32).min)
BIG = 1 << 30

TRACE = False
_CACHE = {}


def _bc_free(ap, rep, where):
    dims = list(ap.ap)
    if where == 'outer':
        new = dims[:-1] + [[0, rep]] + dims[-1:]
    else:
        new = dims + [[0, rep]]
    return bass.AP(ap.tensor, ap.offset, new)



def _split_oversized_waits(nc_, max_waits=1):
    """walrus codegen allows only a limited number of sem-waits per
    instruction; move overflow waits onto preceding same-engine drains."""
    n_split = 0
    for f in nc_.m.functions:
        for bb in f.blocks:
            new_insts = []
            for inst in bb.instructions:
                si = inst.sync_info
                if si is not None and si.on_wait and len(si.on_wait) > max_waits:
                    waits = list(si.on_wait)
                    overflow, keep = waits[:-max_waits], waits[-max_waits:]
                    k = 0
                    while overflow:
                        chunk, overflow = overflow[:max_waits], overflow[max_waits:]
                        d = mybir.InstDrain(name=f"{inst.name}-wsplit{k}", ins=[],
                                            outs=[], bass_is_fusable=False)
                        d.engine = inst.engine
                        d.sync_info = mybir.SyncInfo(on_wait=chunk, on_update=[])
                        new_insts.append(d)
                        k += 1
                        n_split += 1
                    inst.sync_info = mybir.SyncInfo(on_wait=keep,
                                                    on_update=list(si.on_update))
                new_insts.append(inst)
            bb.instructions = new_insts
    return n_split


def _pbcast(nc, out_ap, row_ap):
    """Broadcast a [1, F] SBUF row to [P, F] via a replicated-read DMA."""
    rep = bass.AP(row_ap.tensor, row_ap.offset,
                  [list(row_ap.ap[0]), [0, out_ap.shape[0]], list(row_ap.ap[-1])])
    return nc.sync.dma_start(out_ap, rep)


def build_nc():
    nc = bass.Bass()
    dp = nc.declare_dram_parameter

    emb0_in = dp("emb0", [128, NKD * TOK], f32, isOutput=False)
    sig_in = dp("sigma1", [1, 1], f32, isOutput=False)
    frq_in = dp("freqs", [128, 1], f32, isOutput=False)
    tw1_in = dp("te_w1p", [128, 2 * 128], f32, isOutput=False)
    tb1_in = dp("te_b1", [128, 1], f32, isOutput=False)
    tw2_in = dp("te_w2p", [128, 128], f32, isOutput=False)
    tb2_in = dp("te_b2", [128, 1], f32, isOutput=False)
    adw_in = dp("adaw", [128, 5 * 18 * 128], f32, isOutput=False)
    adb_in = dp("adab", [128, 5 * 18], f32, isOutput=False)
    wuz_in = dp("win_uz", [128, NL * 12 * NKD * 128], f32, isOutput=False)
    wde_in = dp("wdelta", [128, NL * 6 * NKD * 128], f32, isOutput=False)
    wbc_in = dp("wbc", [128, NL * NKD * 64], f32, isOutput=False)
    dtb_in = dp("dtb", [128, NL * 2 * NPT], f32, isOutput=False)
    ap_in = dp("Ap", [128, NL * 2 * NPT * 16], f32, isOutput=False)
    dsk_in = dp("dsk", [128, NL * 2 * NPT], f32, isOutput=False)
    wo_in = dp("wout", [128, NL * 6 * NKD * 128], f32, isOutput=False)
    w12_in = dp("w12", [128, NL * 6 * NKD * 128], f32, isOutput=False)
    w3_in = dp("w3", [128, NL * 6 * 3 * 128], f32, isOutput=False)
    te_in = dp("temb", [128, NKD * VSH], bf16, isOutput=False)
    mrow_in = dp("mrow", [1, 144], f32, isOutput=False)
    oneg_in = dp("offneg", [128, 8], i32, isOutput=False)
    ofz_in = dp("offz", [128, 8], i32, isOutput=False)
    onc_in = dp("offnc", [128, 8], i32, isOutput=False)

    out_lp = dp("out_lp", [B * TOK + 1, VSH], f32, isOutput=True)

    g4 = [[0, 1, 2, 3], [4, 5, 6, 7]]
    g2 = [[0, 4], [1, 5], [2, 6], [3, 7]]
    g8 = [list(range(8))]

    with tile.TileContext(nc) as tc, ExitStack() as ctx:
        cpool = ctx.enter_context(tc.tile_pool(name="const", bufs=1))
        drpool = ctx.enter_context(tc.tile_pool(name="dram", bufs=1, space="DRAM"))
        psS = ctx.enter_context(tc.tile_pool(name="psS", bufs=2, space="PSUM"))

        ones = cpool.tile([128, 1], f32, name="ones")
        nc.vector.memset(ones[:], 1.0)
        ones_row = cpool.tile([1, 128], f32, name="ones_row")
        nc.vector.memset(ones_row[:], 1.0)
        frq = cpool.tile([128, 1], f32, name="frq")
        nc.sync.dma_start(frq[:], frq_in[:])
        tb1 = cpool.tile([128, 1], f32, name="tb1")
        nc.sync.dma_start(tb1[:], tb1_in[:])
        tb2 = cpool.tile([128, 1], f32, name="tb2")
        nc.sync.dma_start(tb2[:], tb2_in[:])
        tw1 = cpool.tile([128, 256], f32, name="tw1")
        nc.sync.dma_start(tw1[:], tw1_in[:])
        tw2 = cpool.tile([128, 128], f32, name="tw2")
        nc.sync.dma_start(tw2[:], tw2_in[:])
        sigt = cpool.tile([1, 1], f32, name="sigt")
        nc.sync.dma_start(sigt[:], sig_in[:])
        adab = cpool.tile([128, 90], f32, name="adab")
        nc.sync.dma_start(adab[:], adb_in[:])
        dtb = cpool.tile([128, NL * 2 * NPT], f32, name="dtb")
        nc.sync.dma_start(dtb[:], dtb_in[:])
        Apt = cpool.tile([128, NL * 2 * NPT * 16], f32, name="Apt")
        nc.sync.dma_start(Apt[:], ap_in[:])
        dsk = cpool.tile([128, NL * 2 * NPT], f32, name="dsk")
        nc.sync.dma_start(dsk[:], dsk_in[:])
        scg = cpool.tile([128, 90], f32, name="scg")

        # ---- timestep embedder -> cT [128, 1] ----
        sig128 = cpool.tile([128, 1], f32, name="sig128")
        _pbcast(nc, sig128[:], sigt[:])
        pio2 = cpool.tile([128, 1], f32, name="pio2")
        nc.vector.memset(pio2[:], float(np.pi / 2))
        cosv = cpool.tile([128, 1], f32, name="cosv")
        nc.scalar.activation(cosv[:], sig128[:], AF.Sin,
                             bias=pio2[:, 0:1], scale=frq[:, 0:1])
        sinv = cpool.tile([128, 1], f32, name="sinv")
        nc.scalar.activation(sinv[:], sig128[:], AF.Sin, bias=0.0, scale=frq[:, 0:1])
        ps_te = psS.tile([128, 1], f32, tag="small", name="ps_te")
        nc.tensor.matmul(ps_te[:], tw1[:, 0:128], cosv[:], start=True, stop=False)
        nc.tensor.matmul(ps_te[:], tw1[:, 128:256], sinv[:], start=False, stop=True)
        c1 = cpool.tile([128, 1], f32, name="c1")
        nc.scalar.activation(c1[:], ps_te[:], AF.Silu, bias=tb1[:, 0:1], scale=1.0)
        ps_te2 = psS.tile([128, 1], f32, tag="small", name="ps_te2")
        nc.tensor.matmul(ps_te2[:], tw2[:], c1[:], start=True, stop=True)
        cT = cpool.tile([128, 1], f32, name="cT")
        nc.scalar.activation(cT[:], ps_te2[:], AF.Identity, bias=tb2[:, 0:1], scale=1.0)

        # ---- adaLN projections: scg [128, 5*18] ----
        with tc.tile_pool(name="adw", bufs=3) as adwp, \
             tc.tile_pool(name="psA", bufs=2, space="PSUM") as psA:
            for i in range(5):
                wt = adwp.tile([128, 18 * 128], f32, tag="adw", name=f"adw{i}")
                nc.sync.dma_start(wt[:], adw_in[:, i * 18 * 128:(i + 1) * 18 * 128])
                psc = psA.tile([128, 18], f32, tag="psc", name=f"psc{i}")
                for m in range(18):
                    nc.tensor.matmul(psc[:, m:m + 1],
                                     wt[:, m * 128:(m + 1) * 128], cT[:],
                                     start=True, stop=True)
                nc.vector.tensor_add(out=scg[:, i * 18:(i + 1) * 18], in0=psc[:],
                                     in1=adab[:, i * 18:(i + 1) * 18])

        def layernorm(src, dst, scg_i):
            with tc.tile_pool(name=f"ln{scg_i}", bufs=1) as lnp:
                mu_ps = psS.tile([1, TOK], f32, tag="small", name=f"mups{scg_i}")
                s2_ps = psS.tile([1, TOK], f32, tag="small", name=f"s2ps{scg_i}")
                for t in range(NKD):
                    nc.tensor.matmul(mu_ps[:], ones[:],
                                     src[:, t * TOK:(t + 1) * TOK],
                                     start=(t == 0), stop=(t == NKD - 1))
                for t in range(NKD):
                    sq = lnp.tile([128, TOK], f32, tag="lnsq", bufs=2, name=f"sq{scg_i}")
                    nc.scalar.square(sq[:], src[:, t * TOK:(t + 1) * TOK])
                    nc.tensor.matmul(s2_ps[:], ones[:], sq[:],
                                     start=(t == 0), stop=(t == NKD - 1))
                mu = lnp.tile([1, TOK], f32, name=f"mu{scg_i}")
                nc.scalar.activation(mu[:], mu_ps[:], AF.Copy, bias=0.0, scale=1.0 / D)
                musq = lnp.tile([1, TOK], f32, name=f"musq{scg_i}")
                nc.vector.tensor_mul(out=musq[:], in0=mu[:], in1=mu[:])
                vare = lnp.tile([1, TOK], f32, name=f"vare{scg_i}")
                nc.vector.scalar_tensor_tensor(out=vare[:], in0=s2_ps[:],
                                               scalar=1.0 / D, in1=musq[:],
                                               op0=OP.mult, op1=OP.subtract)
                nc.vector.tensor_scalar(out=vare[:], in0=vare[:], scalar1=1e-5,
                                        scalar2=None, op0=OP.add)
                r0 = lnp.tile([1, TOK], f32, name=f"r0{scg_i}")
                nc.vector.reciprocal(r0[:], vare[:])
                rstd = lnp.tile([1, TOK], f32, name=f"rstd{scg_i}")
                nc.scalar.sqrt(rstd[:], r0[:])
                t1 = lnp.tile([1, TOK], f32, name=f"t1{scg_i}")
                nc.vector.tensor_mul(out=t1[:], in0=rstd[:], in1=rstd[:])
                nc.vector.tensor_mul(out=t1[:], in0=t1[:], in1=vare[:])
                nc.vector.tensor_scalar(out=t1[:], in0=t1[:], scalar1=-0.5,
                                        scalar2=1.5, op0=OP.mult, op1=OP.add)
                nc.vector.tensor_mul(out=rstd[:], in0=rstd[:], in1=t1[:])
                nmu = lnp.tile([1, TOK], f32, name=f"nmu{scg_i}")
                nc.vector.scalar_tensor_tensor(out=nmu[:], in0=mu[:], scalar=-1.0,
                                               in1=rstd[:], op0=OP.mult, op1=OP.mult)
                rstd_b = lnp.tile([128, TOK], f32, name=f"rstdb{scg_i}")
                _pbcast(nc, rstd_b[:], rstd[:])
                nmu_b = lnp.tile([128, TOK], f32, name=f"nmub{scg_i}")
                _pbcast(nc, nmu_b[:], nmu[:])
                for t in range(NKD):
                    xh = lnp.tile([128, TOK], f32, tag="lnxh", bufs=2, name=f"xh{scg_i}")
                    nc.vector.tensor_mul(out=xh[:], in0=src[:, t * TOK:(t + 1) * TOK],
                                         in1=rstd_b[:])
                    nc.vector.tensor_add(out=xh[:], in0=xh[:], in1=nmu_b[:])
                    nc.scalar.activation(
                        dst[:, t * TOK:(t + 1) * TOK], xh[:], AF.Identity,
                        bias=scg[:, scg_i * 18 + t:scg_i * 18 + t + 1],
                        scale=scg[:, scg_i * 18 + 6 + t:scg_i * 18 + 7 + t])

        with tc.tile_pool(name="hpool", bufs=1) as hp:
            h = hp.tile([128, NKD * TOK], f32, name="h")
            nc.sync.dma_start(h[:], emb0_in[:])

            for l in range(NL):
                with tc.tile_pool(name=f"trunk{l}", bufs=1) as tp, \
                     tc.tile_pool(name=f"wstr{l}", bufs=3) as wp, \
                     tc.tile_pool(name=f"psB{l}", bufs=4, space="PSUM") as psB, \
                     tc.tile_pool(name=f"psBC{l}", bufs=1, space="PSUM") as psBC:

                    nrm = tp.tile([128, NKD * TOK], f32, tag="nrm", name=f"nrm{l}")
                    layernorm(h, nrm, 2 * l)

                    # B/C for both dirs (kept in PSUM across the dir loop)
                    wtbc = wp.tile([128, NKD * 64], f32, tag="wbc", name=f"wbc{l}")
                    nc.sync.dma_start(wtbc[:], wbc_in[:, l * NKD * 64:(l + 1) * NKD * 64])
                    psbc = psBC.tile([64, TOK], f32, tag="psbc", name=f"psbc{l}")
                    for k in range(NKD):
                        nc.tensor.matmul(psbc[:], wtbc[:, k * 64:(k + 1) * 64],
                                         nrm[:, k * TOK:(k + 1) * TOK],
                                         start=(k == 0), stop=(k == NKD - 1))
                    bc_sb = tp.tile([64, TOK], bf16, tag="bc_sb", name=f"bcsb{l}")
                    nc.scalar.copy(bc_sb[:], psbc[:])

                    r_all = tp.tile([128, 6 * TOK], f32, tag="r_all", name=f"rall{l}")

                    for d_ in range(2):
                        with tc.tile_pool(name=f"sp1_{l}{d_}", bufs=1) as sp1:
                            ut = sp1.tile([128, NPT * TOK], f32, tag="ut", name=f"u{l}{d_}")
                            zt = sp1.tile([128, NPT * TOK], f32, tag="zt", name=f"z{l}{d_}")
                            for m in range(6):
                                kind, sub = ('u', m) if m < 3 else ('z', m - 3)
                                wt = wp.tile([128, NKD * 128], f32, tag="w",
                                             name=f"wuz{l}{d_}{m}")
                                base = (l * 12 + d_ * 6 + m) * NKD * 128
                                nc.sync.dma_start(wt[:], wuz_in[:, base:base + NKD * 128])
                                ps = psB.tile([128, TOK], f32, tag="ps", name=f"psuz{l}{d_}{m}")
                                for k in range(NKD):
                                    nc.tensor.matmul(ps[:], wt[:, k * 128:(k + 1) * 128],
                                                     nrm[:, k * TOK:(k + 1) * TOK],
                                                     start=(k == 0), stop=(k == NKD - 1))
                                dstt = ut if kind == 'u' else zt
                                nc.scalar.copy(dstt[:, sub * TOK:(sub + 1) * TOK], ps[:])
                            det = sp1.tile([128, NPT * TOK], f32, tag="det", name=f"de{l}{d_}")
                            for m in range(3):
                                wt = wp.tile([128, NKD * 128], f32, tag="w",
                                             name=f"wde{l}{d_}{m}")
                                base = (l * 6 + d_ * 3 + m) * NKD * 128
                                nc.sync.dma_start(wt[:], wde_in[:, base:base + NKD * 128])
                                ps = psB.tile([128, TOK], f32, tag="ps", name=f"psde{l}{d_}{m}")
                                for k in range(NKD):
                                    nc.tensor.matmul(ps[:], wt[:, k * 128:(k + 1) * 128],
                                                     nrm[:, k * TOK:(k + 1) * TOK],
                                                     start=(k == 0), stop=(k == NKD - 1))
                                col = (l * 2 + d_) * NPT + m
                                # softplus(x+b) = ln(exp(x+b) + 1); both funcs
                                # live in natural_log_exp_and_others
                                etmp = sp1.tile([128, TOK], f32, tag="etmp",
                                                bufs=2, name=f"etmp{l}{d_}{m}")
                                nc.scalar.activation(etmp[:], ps[:], AF.Exp,
                                                     bias=dtb[:, col:col + 1], scale=1.0)
                                nc.scalar.activation(det[:, m * TOK:(m + 1) * TOK],
                                                     etmp[:], AF.Ln, bias=1.0, scale=1.0)

                            Bbc = sp1.tile([128, N * TOK], bf16, tag="Bbc", name=f"Bbc{l}{d_}")
                            Cbc = sp1.tile([128, N * TOK], bf16, tag="Cbc", name=f"Cbc{l}{d_}")
                            with tc.tile_pool(name=f"rows{l}{d_}", bufs=1) as rp:
                                brow = rp.tile([1, N * TOK], bf16, name=f"br{l}{d_}")
                                nc.sync.dma_start(out=brow[:],
                                                  in_=bc_sb[d_ * 32:d_ * 32 + 16, :])
                                _pbcast(nc, Bbc[:], brow[:])
                                crow = rp.tile([1, N * TOK], bf16, name=f"cr{l}{d_}")
                                nc.sync.dma_start(out=crow[:],
                                                  in_=bc_sb[d_ * 32 + 16:d_ * 32 + 32, :])
                                _pbcast(nc, Cbc[:], crow[:])

                            du = sp1.tile([128, NPT * TOK], f32, tag="du", name=f"du{l}{d_}")
                            nc.vector.tensor_mul(out=du[:], in0=det[:], in1=ut[:])
                            yt = sp1.tile([128, NPT * TOK], f32, tag="yt", name=f"yt{l}{d_}")

                            with tc.tile_pool(name=f"sp2_{l}{d_}", bufs=2) as sp2:
                                for pt in range(NPT):
                                    dsl = det[:, pt * TOK:(pt + 1) * TOK]
                                    dusl = du[:, pt * TOK:(pt + 1) * TOK]
                                    for chn in range(NCH):
                                        W = CH_N * TOK
                                        dAa = sp2.tile([128, W], f32, tag="bgA",
                                                       name=f"dAa{l}{d_}")
                                        acol = (l * 2 + d_) * NPT * 16 + pt * 16 + chn * CH_N
                                        nc.gpsimd.tensor_tensor(
                                            out=dAa[:], in0=_bc_free(dsl, CH_N, 'outer'),
                                            in1=_bc_free(Apt[:, acol:acol + CH_N], TOK, 'inner'),
                                            op=OP.mult)
                                        dA = sp2.tile([128, W], f32, tag="bgB",
                                                      name=f"dA{l}{d_}")
                                        nc.scalar.activation(dA[:], dAa[:], AF.Exp)
                                        dBu = sp2.tile([128, W], f32, tag="bgC",
                                                       name=f"dBu{l}{d_}")
                                        nc.vector.tensor_tensor(
                                            out=dBu[:], in0=_bc_free(dusl, CH_N, 'outer'),
                                            in1=Bbc[:, chn * W:(chn + 1) * W], op=OP.mult)
                                        hs = sp2.tile([128, W], bf16, tag="bgD",
                                                      name=f"hs{l}{d_}")
                                        for j in range(CH_N):
                                            o_ = hs[:, j * TOK:(j + 1) * TOK]
                                            a_ = dA[:, j * TOK:(j + 1) * TOK]
                                            b_ = dBu[:, j * TOK:(j + 1) * TOK]
                                            if d_ == 1:
                                                o_, a_, b_ = o_[:, ::-1], a_[:, ::-1], b_[:, ::-1]
                                            nc.vector.tensor_tensor_scan(
                                                o_, a_, b_, 0.0, op0=OP.mult, op1=OP.add)
                                        hc = sp2.tile([128, W], bf16, tag="bgAh",
                                                      name=f"hc{l}{d_}")
                                        nc.vector.tensor_tensor(
                                            out=hc[:], in0=hs[:],
                                            in1=Cbc[:, chn * W:(chn + 1) * W], op=OP.mult)
                                        ysl = yt[:, pt * TOK:(pt + 1) * TOK]
                                        if chn == 0:
                                            nc.vector.tensor_reduce(
                                                ysl, hc[:].rearrange("p (n t) -> p t n", n=CH_N),
                                                axis=mybir.AxisListType.X, op=OP.add)
                                        else:
                                            ytmp = sp2.tile([128, TOK], f32, tag="ytmp",
                                                            name=f"ytmp{l}{d_}")
                                            nc.vector.tensor_reduce(
                                                ytmp[:], hc[:].rearrange("p (n t) -> p t n", n=CH_N),
                                                axis=mybir.AxisListType.X, op=OP.add)
                                            nc.vector.tensor_add(out=ysl, in0=ysl, in1=ytmp[:])
                                    col = (l * 2 + d_) * NPT + pt
                                    yD = sp2.tile([128, TOK], f32, tag="yD", name=f"yD{l}{d_}")
                                    nc.vector.scalar_tensor_tensor(
                                        out=yD[:], in0=ut[:, pt * TOK:(pt + 1) * TOK],
                                        scalar=dsk[:, col:col + 1],
                                        in1=yt[:, pt * TOK:(pt + 1) * TOK],
                                        op0=OP.mult, op1=OP.add)
                                    sz = sp2.tile([128, TOK], f32, tag="sz", name=f"sz{l}{d_}")
                                    nc.scalar.activation(sz[:], zt[:, pt * TOK:(pt + 1) * TOK],
                                                         AF.Silu)
                                    nc.vector.tensor_mul(
                                        out=r_all[:, (d_ * NPT + pt) * TOK:(d_ * NPT + pt + 1) * TOK],
                                        in0=yD[:], in1=sz[:])

                    # --- Wout partials + AllReduce + residual ---
                    arin = drpool.tile([D, TOK], f32, name=f"arin{l}s")
                    for m in range(NKD):
                        wt = wp.tile([128, NKD * 128], f32, tag="w", name=f"wo{l}{m}")
                        base = (l * 6 + m) * NKD * 128
                        nc.sync.dma_start(wt[:], wo_in[:, base:base + NKD * 128])
                        ps = psB.tile([128, TOK], f32, tag="ps", name=f"pso{l}{m}")
                        for k in range(NKD):
                            nc.tensor.matmul(ps[:], wt[:, k * 128:(k + 1) * 128],
                                             r_all[:, k * TOK:(k + 1) * TOK],
                                             start=(k == 0), stop=(k == NKD - 1))
                        arc = tp.tile([128, TOK], f32, tag="arcp", bufs=2,
                                      name=f"arc{l}{m}")
                        nc.scalar.copy(arc[:], ps[:])
                        nc.sync.dma_start(arin[m * 128:(m + 1) * 128, :], arc[:])
                    arout = drpool.tile([D, TOK], f32, name=f"arout{l}s")
                    nc.gpsimd.collective_compute(
                        "AllReduce", OP.add, replica_groups=g4,
                        ins=[arin[:].opt()], outs=[arout[:].opt()])
                    for t in range(NKD):
                        sso = tp.tile([128, TOK], f32, tag="sso", bufs=2, name=f"sso{l}{t}")
                        nc.sync.dma_start(sso[:], arout[t * 128:(t + 1) * 128, :])
                        nc.vector.scalar_tensor_tensor(
                            out=h[:, t * TOK:(t + 1) * TOK], in0=sso[:],
                            scalar=scg[:, 2 * l * 18 + 12 + t:2 * l * 18 + 13 + t],
                            in1=h[:, t * TOK:(t + 1) * TOK], op0=OP.mult, op1=OP.add)

                    # --- adaln2 + MLP ---
                    nrm2 = tp.tile([128, NKD * TOK], f32, tag="nrm", name=f"nrm2{l}")
                    layernorm(h, nrm2, 2 * l + 1)
                    gt = tp.tile([128, 3 * TOK], f32, tag="gt", name=f"gt{l}")
                    for i in range(3):
                        wt1 = wp.tile([128, NKD * 128], f32, tag="w", name=f"w1{l}{i}")
                        base = (l * 6 + i) * NKD * 128
                        nc.sync.dma_start(wt1[:], w12_in[:, base:base + NKD * 128])
                        ps1 = psB.tile([128, TOK], f32, tag="ps", name=f"psm1{l}{i}")
                        for k in range(NKD):
                            nc.tensor.matmul(ps1[:], wt1[:, k * 128:(k + 1) * 128],
                                             nrm2[:, k * TOK:(k + 1) * TOK],
                                             start=(k == 0), stop=(k == NKD - 1))
                        wt2 = wp.tile([128, NKD * 128], f32, tag="w", name=f"w2{l}{i}")
                        base = (l * 6 + 3 + i) * NKD * 128
                        nc.sync.dma_start(wt2[:], w12_in[:, base:base + NKD * 128])
                        ps2 = psB.tile([128, TOK], f32, tag="ps", name=f"psm2{l}{i}")
                        for k in range(NKD):
                            nc.tensor.matmul(ps2[:], wt2[:, k * 128:(k + 1) * 128],
                                             nrm2[:, k * TOK:(k + 1) * TOK],
                                             start=(k == 0), stop=(k == NKD - 1))
                        sa = tp.tile([128, TOK], f32, tag="sa", bufs=2, name=f"sa{l}{i}")
                        nc.scalar.activation(sa[:], ps1[:], AF.Silu)
                        nc.vector.tensor_mul(out=gt[:, i * TOK:(i + 1) * TOK],
                                             in0=sa[:], in1=ps2[:])
                    arin2 = drpool.tile([D, TOK], f32, name=f"arin{l}m")
                    for m in range(NKD):
                        wt = wp.tile([128, 3 * 128], f32, tag="w3", name=f"w3_{l}{m}")
                        base = (l * 6 + m) * 3 * 128
                        nc.sync.dma_start(wt[:], w3_in[:, base:base + 3 * 128])
                        ps = psB.tile([128, TOK], f32, tag="ps", name=f"psm3{l}{m}")
                        for k in range(3):
                            nc.tensor.matmul(ps[:], wt[:, k * 128:(k + 1) * 128],
                                             gt[:, k * TOK:(k + 1) * TOK],
                                             start=(k == 0), stop=(k == 2))
                        arc2 = tp.tile([128, TOK], f32, tag="arcp", bufs=2,
                                       name=f"arc2{l}{m}")
                        nc.scalar.copy(arc2[:], ps[:])
                        nc.sync.dma_start(arin2[m * 128:(m + 1) * 128, :], arc2[:])
                    arout2 = drpool.tile([D, TOK], f32, name=f"arout{l}m")
                    nc.gpsimd.collective_compute(
                        "AllReduce", OP.add, replica_groups=g4,
                        ins=[arin2[:].opt()], outs=[arout2[:].opt()])
                    for t in range(NKD):
                        mlo = tp.tile([128, TOK], f32, tag="sso", bufs=2, name=f"mlo{l}{t}")
                        nc.sync.dma_start(mlo[:], arout2[t * 128:(t + 1) * 128, :])
                        nc.vector.scalar_tensor_tensor(
                            out=h[:, t * TOK:(t + 1) * TOK], in0=mlo[:],
                            scalar=scg[:, (2 * l + 1) * 18 + 12 + t:(2 * l + 1) * 18 + 13 + t],
                            in1=h[:, t * TOK:(t + 1) * TOK], op0=OP.mult, op1=OP.add)

            # final adaLN -> bf16, AllGather across batch pairs
            agin = drpool.tile([D, TOK], bf16, name="agin")
            with tc.tile_pool(name="fin", bufs=1) as fp:
                nrm_f = fp.tile([128, NKD * TOK], bf16, name="nrm_f")
                layernorm(h, nrm_f, 4)
                for t in range(NKD):
                    nc.sync.dma_start(agin[t * 128:(t + 1) * 128, :],
                                      nrm_f[:, t * TOK:(t + 1) * TOK])
            agout = drpool.tile([2 * D, TOK], bf16, name="agout")
            nc.gpsimd.collective_compute(
                "AllGather", OP.bypass, replica_groups=g2,
                ins=[agin[:].opt()], outs=[agout[:].opt()])

        # ================= lm_head =================
        with tc.tile_pool(name="lm", bufs=1) as lp_, \
             tc.tile_pool(name="psL", bufs=4, space="PSUM") as psL:
            nrm_all = lp_.tile([128, NKD * 2 * TOK], bf16, name="nrm_all")
            for b_ in range(2):
                for t in range(NKD):
                    nc.sync.dma_start(
                        nrm_all[:, t * 2 * TOK + b_ * TOK:t * 2 * TOK + (b_ + 1) * TOK],
                        agout[b_ * D + t * 128:b_ * D + (t + 1) * 128, :])
            temb = lp_.tile([128, NKD * VSH], bf16, name="temb")
            for k in range(NKD):
                nc.sync.dma_start(temb[:, k * VSH:(k + 1) * VSH],
                                  te_in[:, k * VSH:(k + 1) * VSH])

            mrow_t = lp_.tile([1, 144], f32, name="mrow_t")
            nc.sync.dma_start(mrow_t[:], mrow_in[:])
            offneg = lp_.tile([128, 8], i32, name="offneg")
            nc.sync.dma_start(offneg[:], oneg_in[:])
            offz = lp_.tile([128, 8], i32, name="offz")
            nc.sync.dma_start(offz[:], ofz_in[:])
            offnc = lp_.tile([128, 8], i32, name="offnc")
            nc.sync.dma_start(offnc[:], onc_in[:])

            negsm = lp_.tile([128, VSH], f32, name="negsm")
            nc.vector.memset(negsm[:], NEG)
            zcol = lp_.tile([128, 1], f32, name="zcol")
            nc.vector.memset(zcol[:], 0.0)
            ncol = lp_.tile([128, 1], f32, name="ncol")
            nc.vector.memset(ncol[:], NEG)

            lpdmas = []
            for blk in range(8):
                esb = lp_.tile([128, VSH], f32, tag="esb", bufs=1, name=f"esb{blk}")
                c0 = 0
                for vt, w in enumerate(VT_W):
                    ps = psL.tile([128, 512], f32, tag="pslm", name=f"pslm{blk}{vt}")
                    last_vt = (vt == len(VT_W) - 1)
                    for k in range(NKD):
                        nc.tensor.matmul(
                            ps[:, :w],
                            nrm_all[:, k * 2 * TOK + blk * 128:k * 2 * TOK + (blk + 1) * 128],
                            temb[:, k * VSH + c0:k * VSH + c0 + w],
                            start=(k == 0), stop=(k == NKD - 1 and not last_vt))
                    if last_vt:
                        # force the MASK_ID logit to -1e5 (only nonzero on the
                        # core owning that vocab column)
                        nc.tensor.matmul(ps[:, :w], ones_row[:, 0:128],
                                         mrow_t[:, :w], start=False, stop=True)
                    nc.scalar.activation(esb[:, c0:c0 + w], ps[:, :w], AF.Exp)
                    c0 += w
                sl = lp_.tile([128, 1], f32, tag="sl", bufs=2, name=f"sl{blk}")
                nc.vector.tensor_reduce(sl[:], esb[:],
                                        axis=mybir.AxisListType.X, op=OP.add)
                lmin = drpool.tile([128, 1], f32, name=f"lmin{blk}")
                nc.sync.dma_start(lmin[:], sl[:])
                lmout = drpool.tile([128, 1], f32, name=f"lmout{blk}")
                nc.gpsimd.collective_compute(
                    "AllReduce", OP.add, replica_groups=g8,
                    ins=[lmin[:].opt()], outs=[lmout[:].opt()])
                gs = lp_.tile([128, 1], f32, tag="gs", bufs=2, name=f"gs{blk}")
                nc.sync.dma_start(gs[:], lmout[:])
                rec = lp_.tile([128, 1], f32, tag="rec", bufs=2, name=f"rec{blk}")
                nc.vector.reciprocal(rec[:], gs[:])
                lpt = lp_.tile([128, VSH], f32, tag="lpt", bufs=1, name=f"lpt{blk}")
                c0 = 0
                for vt, w in enumerate(VT_W):
                    nc.scalar.activation(lpt[:, c0:c0 + w], esb[:, c0:c0 + w],
                                         AF.Ln, bias=0.0, scale=rec[:, 0:1])
                    c0 += w
                dmains = nc.sync.dma_start(out_lp[blk * 128:(blk + 1) * 128, :], lpt[:])
                lpdmas.append(dmains)

            # --- forced-row overwrites ---
            flat = bass.AP(out_lp[:].tensor, 0, [[1, (B * TOK + 1) * VSH], [1, 1]])
            negin = negsm[:]
            for blk in range(8):
                s1 = nc.gpsimd.indirect_dma_start(
                    out=out_lp[:],
                    out_offset=bass.IndirectOffsetOnAxis(ap=offneg[:, blk:blk + 1], axis=0),
                    in_=negin, in_offset=None)
                add_dep_helper(s1.ins, lpdmas[blk].ins, sync=True,
                               reason="neg rows after lp rows")
                s2 = nc.gpsimd.indirect_dma_start(
                    out=flat,
                    out_offset=bass.IndirectOffsetOnAxis(ap=offz[:, blk:blk + 1], axis=0),
                    in_=zcol[:], in_offset=None)
                add_dep_helper(s2.ins, s1.ins, sync=True, reason="zeros after neg rows")
                s3 = nc.gpsimd.indirect_dma_start(
                    out=flat,
                    out_offset=bass.IndirectOffsetOnAxis(ap=offnc[:, blk:blk + 1], axis=0),
                    in_=ncol[:], in_offset=None)
                add_dep_helper(s3.ins, s1.ins, sync=True, reason="negcol after neg rows")
                add_dep_helper(s3.ins, lpdmas[blk].ins, sync=True,
                               reason="negcol after lp rows")

    _split_oversized_waits(nc)
    return nc


# ================= host side =================

def _pack_lhsT(w, nk, nm, mtile):
    """w [K, M] -> [128, nm*nk*mtile], col ((m*nk + k)*mtile + j) = w[k*128+p, m*mtile+j]."""
    K, M = w.shape
    assert K == nk * 128 and M == nm * mtile, (w.shape, nk, nm, mtile)
    arr = np.ascontiguousarray(w).reshape(nk, 128, nm, mtile)
    return np.ascontiguousarray(arr.transpose(1, 2, 0, 3)).reshape(128, nm * nk * mtile)


def _fm(x, ntiles):
    """x [tok, D] -> feature-major [128, ntiles*tok]."""
    tok, Dd = x.shape
    assert Dd == ntiles * 128
    return np.ascontiguousarray(
        np.ascontiguousarray(x.T).reshape(ntiles, 128, tok).transpose(1, 0, 2)
    ).reshape(128, -1)


def _pp(v, groups):
    """v [groups*128] -> per-partition cols [128, groups]."""
    return np.ascontiguousarray(np.ascontiguousarray(v).reshape(groups, 128).T)


def _blk8(v):
    """v [1024] -> [128, 8] with col j = tokens j*128..(j+1)*128."""
    return np.ascontiguousarray(v.reshape(8, 128).T)


def kernel(**inputs):
    inp = {k: np.asarray(v) for k, v in inputs.items()}
    x_t = np.asarray(inp['x_t']).astype(np.int64)
    f = {k: inp[k].astype(np.float64) for k in inp if k != 'x_t'}

    if 'nc' not in _CACHE:
        _CACHE['nc'] = build_nc()
    nc = _CACHE['nc']

    tok_emb = inp['tok_emb'].astype(np.float32)
    pos_emb = inp['pos_emb'].astype(np.float32)

    half = FREQ // 2
    freqs = np.exp(-np.log(10000.0) * np.arange(half, dtype=np.float64) / half)

    Win, Wx, Wdt = f['Win'], f['Wx'], f['Wdt']
    wdelta_full = np.zeros((NL, 2, D, DI))
    wbc_full = np.zeros((NL, 2, D, 2 * N))
    for l in range(NL):
        for d_ in range(2):
            wu = Win[l, d_][:, :DI]
            wdelta_full[l, d_] = (wu @ Wx[l, d_][:, :DTR]) @ Wdt[l, d_]
            wbc_full[l, d_] = wu @ Wx[l, d_][:, DTR:]
    A_full = -np.exp(f['A_log'])

    ada_ws = [f['adaln1_w'][0], f['adaln2_w'][0], f['adaln1_w'][1],
              f['adaln2_w'][1], f['outadaln_w']]
    ada_bs = [f['adaln1_b'][0], f['adaln2_b'][0], f['adaln1_b'][1],
              f['adaln2_b'][1], f['outadaln_b']]
    adaw_p = np.concatenate(
        [_pack_lhsT(w.astype(np.float32), 1, 18, 128) for w in ada_ws], axis=1)
    adab_cols = []
    for bvec in ada_bs:
        bb = bvec.copy()
        bb[D:2 * D] += 1.0
        adab_cols.append(_pp(bb.astype(np.float32), 18))
    adab_p = np.concatenate(adab_cols, axis=1)

    te_w1p = _pack_lhsT(f['te_w1'].astype(np.float32), 2, 1, 128)

    x_flat = x_t.reshape(-1)
    tvec = np.arange(B * TOK, dtype=np.int64)
    mskf = x_flat == MASK_ID

    in_maps = []
    for c in range(NCORES):
        b = c // 4
        g = c % 4
        v0 = c * VSH

        emb = tok_emb[x_t[b]] + pos_emb[:L]
        m = {
            'emb0': _fm(emb, NKD).astype(np.float32),
            'sigma1': np.array([[inp['sigma'][b]]], dtype=np.float32),
            'freqs': freqs.astype(np.float32).reshape(half, 1),
            'te_w1p': te_w1p.astype(np.float32),
            'te_b1': inp['te_b1'].astype(np.float32).reshape(COND, 1),
            'te_w2p': inp['te_w2'].astype(np.float32),
            'te_b2': inp['te_b2'].astype(np.float32).reshape(COND, 1),
            'adaw': adaw_p.astype(np.float32),
            'adab': adab_p.astype(np.float32),
        }
        wuz_cols = []
        for l in range(NL):
            for d_ in range(2):
                uq = Win[l, d_][:, g * NQ:(g + 1) * NQ]
                zq = Win[l, d_][:, DI + g * NQ:DI + (g + 1) * NQ]
                wuz_cols.append(_pack_lhsT(
                    np.concatenate([uq, zq], axis=1).astype(np.float32), NKD, 6, 128))
        m['win_uz'] = np.concatenate(wuz_cols, axis=1)

        wde_cols, wo_cols, ap_cols, dtb_cols, dsk_cols = [], [], [], [], []
        for l in range(NL):
            for d_ in range(2):
                wq = wdelta_full[l, d_][:, g * NQ:(g + 1) * NQ]
                wde_cols.append(_pack_lhsT(wq.astype(np.float32), NKD, 3, 128))
                Aq = A_full[l, d_][g * NQ:(g + 1) * NQ, :]
                ap_cols.append(np.ascontiguousarray(
                    Aq.reshape(3, 128, 16).transpose(1, 0, 2)).reshape(128, 48
                    ).astype(np.float32))
                dtb_cols.append(_pp(f['dt_bias'][l, d_][g * NQ:(g + 1) * NQ]
                                    .astype(np.float32), 3))
                dsk_cols.append(_pp(f['Dskip'][l, d_][g * NQ:(g + 1) * NQ]
                                    .astype(np.float32), 3))
            wo_rows = np.concatenate(
                [f['Wout'][l, 0][g * NQ:(g + 1) * NQ, :],
                 f['Wout'][l, 1][g * NQ:(g + 1) * NQ, :]], axis=0)
            wo_cols.append(_pack_lhsT(wo_rows.astype(np.float32), 6, 6, 128))
        m['wdelta'] = np.concatenate(wde_cols, axis=1)
        m['wout'] = np.concatenate(wo_cols, axis=1)
        m['Ap'] = np.concatenate(ap_cols, axis=1)
        m['dtb'] = np.concatenate(dtb_cols, axis=1)
        m['dsk'] = np.concatenate(dsk_cols, axis=1)

        wbc_cols = []
        for l in range(NL):
            wbc_all = np.concatenate([wbc_full[l, 0], wbc_full[l, 1]], axis=1)
            wbc_cols.append(_pack_lhsT(wbc_all.astype(np.float32), NKD, 1, 64))
        m['wbc'] = np.concatenate(wbc_cols, axis=1)

        w12_cols, w3_cols = [], []
        for l in range(NL):
            w1q = f['mlp_w1'][l][:, g * HQ:(g + 1) * HQ]
            w2q = f['mlp_w2'][l][:, g * HQ:(g + 1) * HQ]
            w12_cols.append(_pack_lhsT(
                np.concatenate([w1q, w2q], axis=1).astype(np.float32), NKD, 6, 128))
            w3q = f['mlp_w3'][l][g * HQ:(g + 1) * HQ, :]
            w3_cols.append(_pack_lhsT(w3q.astype(np.float32), 3, 6, 128))
        m['w12'] = np.concatenate(w12_cols, axis=1)
        m['w3'] = np.concatenate(w3_cols, axis=1)

        te_sh = tok_emb[v0:v0 + VSH, :].astype(ml_dtypes.bfloat16)
        m['temb'] = np.ascontiguousarray(
            np.ascontiguousarray(te_sh.T).reshape(NKD, 128, VSH).transpose(1, 0, 2)
        ).reshape(128, -1)

        DUMP_ROW = B * TOK
        DUMP_EL = B * TOK * VSH
        offneg = np.where(mskf, DUMP_ROW, tvec)
        valid = (~mskf) & (x_flat >= v0) & (x_flat < v0 + VSH)
        offz = np.where(valid, tvec * VSH + (x_flat - v0), DUMP_EL)
        if v0 <= MASK_ID < v0 + VSH:
            offnc = tvec * VSH + (MASK_ID - v0)
        else:
            offnc = np.full(B * TOK, DUMP_EL, dtype=np.int64)
        mrow = np.zeros((1, 144), dtype=np.float32)
        if v0 <= MASK_ID < v0 + VSH:
            mrow[0, (MASK_ID - v0) - 12 * 512] = -1e5
        m['mrow'] = mrow
        m['offneg'] = _blk8(offneg).astype(np.int32)
        m['offz'] = _blk8(offz).astype(np.int32)
        m['offnc'] = _blk8(offnc).astype(np.int32)
        in_maps.append(m)

    res = run_bass_kernel_spmd(nc, in_maps, core_ids=list(range(NCORES)),
                               trace=TRACE)
    _CACHE['last_result'] = res

    out = np.empty((B, L, V), dtype=np.float32)
    for c in range(NCORES):
        o = res.results[c]['out_lp']
        out[0, :, c * VSH:(c + 1) * VSH] = o[:L]
        out[1, :, c * VSH:(c + 1) * VSH] = o[L:2 * L]
    return out

